# revision 33
# baseline (speedup 1.0000x reference)
"""Trainium2 Bass kernel for the CorticalMicrocircuit RHS evaluation.

Computes dy = f(y; params, drive) for B=2,097,152 independent 12-state
neural-mass circuits, data-parallel across 8 NeuronCores (shard batch,
replicate scalars).

Math per batch element (all scalar coefficients precomputed on host):
  out[i]   = y[6+i]                                          i in 0..5
  out[6+i] = K_i + E_i*y[i] + D_i*y[6+i] + sum_j C_ij * G_j
  G_j      = sigmoid(r_j*y[j] - r_j*v0_j)       (2*e0_j folded into C_ij)

Device mapping per core (B_shard = 262144 = 128 partitions x 2048 f32):
  - ACT: per row h_i = E_i*v_i + K_i (affine init, written to the PSUM
    accumulator for DVE rows), then 6 sigmoids computed IN-PLACE over the
    state tiles (dead after the inits read them).
  - DVE: FMA chains acc = src*c + acc with the accumulator in PSUM, so
    each op reads one SBUF operand on the dedicated port -> the shared
    DVE/GpSimd SBUF port pair stays free for GPSIMD.
  - GPSIMD: rows 2 and 5 (mul on tensor_scalar + add on tensor_tensor,
    all in SBUF) run concurrently on the shared port pair.
  - All DMA on HWDGE (nc.sync). Output rows 0..5 are SBUF->DRAM copies
    of the loaded state tiles; computed rows write back over dead input
    tiles to save SBUF.
"""

import numpy as np

import concourse.bacc as bacc
import concourse.mybir as mybir
import concourse.tile as tile
from concourse.bass_utils import run_bass_kernel_spmd

F32 = mybir.dt.float32

# ---- problem geometry (hardcoded per contest contract) ----
B = 2097152
NCORES = 8
BS = B // NCORES          # 262144 per core
P = 128
FTOT = BS // P            # 2048 free elements per partition
NROW = 12
DT = np.float32(0.002)
NT = 50000

# Per output row i: the FMA terms. ('d', i) is the damping term D_i*y[6+i]
# (always first); ('g', j) reads sigmoid tile G_j.
TERMS = [
    [('d', 0), ('g', 2), ('g', 0), ('g', 3), ('g', 4)],
    [('d', 1), ('g', 2), ('g', 0), ('g', 1), ('g', 3), ('g', 4)],
    [('d', 2), ('g', 0), ('g', 1), ('g', 3)],
    [('d', 3), ('g', 0), ('g', 1), ('g', 2), ('g', 3), ('g', 4), ('g', 5)],
    [('d', 4), ('g', 0), ('g', 1), ('g', 2), ('g', 3), ('g', 4), ('g', 5)],
    [('d', 5), ('g', 0), ('g', 3), ('g', 4), ('g', 5)],
]
# coupling param index + sign per 'g' term, aligned with TERMS[i] g-order.
G_COUP = [
    [(18, 1), (0, 1), (27, -1), (37, -1)],
    [(19, 1), (1, 1), (10, 1), (28, -1), (38, -1)],
    [(2, 1), (11, 1), (29, -1)],
    [(3, 1), (12, 1), (21, 1), (30, -1), (39, -1), (46, -1)],
    [(4, 1), (13, 1), (22, 1), (31, -1), (40, -1), (47, -1)],
    [(5, 1), (32, -1), (41, -1), (48, -1)],
]
# param index tables (reference.py unpack order)
GAIN = [6, 14, 23, 33, 42, 49]      # a_p23, a_p5, a_st, b_PV, b_SST, b_VIP
E0 = [7, 15, 24, 34, 43, 50]
V0 = [8, 16, 25, 35, 44, 51]
RR = [9, 17, 26, 36, 45, 52]
NN = [53, 54, 55, 56, 57, 58]

# coef column layout
def _sig_scale_col(j): return 2 * j
def _sig_bias_col(j): return 2 * j + 1
def _d_col(i): return 12 + 2 * i      # D_i = -2*gain_i
def _k_col(i): return 13 + 2 * i      # K_i = drive_i
def _e_col(i): return 24 + i          # E_i = -gain_i^2
_G_COL0 = 30
_g_col_of = {}
_c = _G_COL0
for _i in range(6):
    for _g in range(len(G_COUP[_i])):
        _g_col_of[(_i, _g)] = _c
        _c += 1
NCOEF = 64
assert _c <= NCOEF

# row -> number of leading FMA terms executed on GPSIMD (rest on DVE).
# GPSIMD terms run as ACT mul (tensor_scalar w/ scalar-AP is ~12x slow on
# the Q7 ucode) + GPSIMD tensor_tensor add.
POOL_PLAN = {2: 4, 5: 5}


def _build(ftot=FTOT, w=1024, pool_plan=POOL_PLAN):
    """Build the SPMD program (identical on every core). Bacc (not plain
    Bass): its finalize() splits multi-sem waits into EventSemaphore
    instructions — this walrus accepts at most ONE sync-wait per
    compute/DMA instruction."""
    nc = bacc.Bacc()
    y = nc.declare_dram_parameter("y", [NROW, P, ftot], F32, isOutput=False)
    coef = nc.declare_dram_parameter("coef", [P, NCOEF], F32, isOutput=False)
    out = nc.declare_dram_parameter("out", [NROW, P, ftot], F32, isOutput=True)
    nchunk = ftot // w
    banks_per_acc = (4 * w + 2047) // 2048
    # every row whose DVE tail has >=2 ops holds a PSUM accumulator
    n_psum_rows = sum(1 for i in range(6)
                      if len(TERMS[i]) - pool_plan.get(i, 0) >= 2)
    psum_bufs = max(1, 8 // max(1, n_psum_rows * banks_per_acc))

    mult = mybir.AluOpType.mult
    add = mybir.AluOpType.add
    Sig = mybir.ActivationFunctionType.Sigmoid
    Ident = mybir.ActivationFunctionType.Identity

    with tile.TileContext(nc) as tc:
        with (
            tc.tile_pool(name="const", bufs=1) as cpool,
            tc.tile_pool(name="io", bufs=2) as iop,
            tc.tile_pool(name="mid", bufs=2) as midp,
            tc.tile_pool(name="acc", bufs=1, space="PSUM") as ppool,
        ):
            ct = cpool.tile([P, NCOEF], F32)
            nc.sync.dma_start(out=ct[:], in_=coef[:, :])
            # one tiny coef read per compute engine right after the load:
            # advances each engine's clock past the coef DMA lane so real
            # ops don't each need an extra wait on it.
            scr = cpool.tile([P, 4], F32)
            nc.scalar.copy(scr[:, 0:1], ct[:, 0:1])
            nc.vector.tensor_copy(scr[:, 1:2], ct[:, 0:1])
            nc.gpsimd.tensor_copy(scr[:, 2:3], ct[:, 0:1])

            def col(c0):
                return ct[:, c0:c0 + 1]

            for c in range(nchunk):
                sl = slice(c * w, (c + 1) * w)
                # one merged DMA per 6-row group: 3 MB transfers, far fewer
                # dispatches and cross-engine semaphores than per-row DMAs
                vm = iop.tile([P, 6 * w], F32, tag="vm", bufs=2)
                dm = iop.tile([P, 6 * w], F32, tag="dm", bufs=2)
                if c == 0:
                    # first chunk: load in 3-row halves so ACT starts as
                    # soon as the first 1.5 MB lands (shorter pipe fill)
                    for lo in (0, 3):
                        nc.sync.dma_start(
                            out=vm[:, lo * w:(lo + 3) * w].rearrange(
                                "p (r w) -> p r w", r=3),
                            in_=y[lo:lo + 3, :, sl].rearrange(
                                "r p w -> p r w"))
                        nc.sync.dma_start(
                            out=dm[:, lo * w:(lo + 3) * w].rearrange(
                                "p (r w) -> p r w", r=3),
                            in_=y[6 + lo:9 + lo, :, sl].rearrange(
                                "r p w -> p r w"))
                else:
                    nc.sync.dma_start(
                        out=vm[:].rearrange("p (r w) -> p r w", r=6),
                        in_=y[0:6, :, sl].rearrange("r p w -> p r w"))
                    nc.sync.dma_start(
                        out=dm[:].rearrange("p (r w) -> p r w", r=6),
                        in_=y[6:12, :, sl].rearrange("r p w -> p r w"))
                # out rows 0..5 are a plain copy of state rows 6..11
                # (ACT's HWDGE ring, so stores don't queue behind loads)
                nc.scalar.dma_start(
                    out=out[0:6, :, sl].rearrange("r p w -> p r w"),
                    in_=dm[:].rearrange("p (r w) -> p r w", r=6))
                vg = [vm[:, j * w:(j + 1) * w] for j in range(6)]
                dts = [dm[:, i * w:(i + 1) * w] for i in range(6)]

                # h_i = E_i*v_i + K_i (ACT, into SBUF). Emission order =
                # demand order: DVE rows' h first so their d-terms start
                # early; all h before any in-place sigmoid (h reads v).
                accs = [None] * 6
                h_order = [i for i in range(6) if pool_plan.get(i, 0) == 0]
                h_order += [i for i in range(6) if pool_plan.get(i, 0) > 0]
                for i in h_order:
                    if pool_plan.get(i, 0) > 0:
                        h = midp.tile([P, w], F32, tag=f"hs{i}", bufs=2,
                                      name=f"hs{i}")
                    else:
                        # DVE rows: init straight into the PSUM accumulator
                        h = ppool.tile([P, w], F32, tag=f"hp{i}",
                                       bufs=psum_bufs, name=f"hp{i}")
                    nc.scalar.activation(h[:], vg[i][:], Ident,
                                         bias=col(_k_col(i)),
                                         scale=col(_e_col(i)))
                    accs[i] = h
                # sigmoids, in place over the (now dead) state tiles,
                # ordered by first use in the DVE rows' chains
                for j in (2, 0, 1, 3, 4, 5):
                    nc.scalar.activation(vg[j][:], vg[j][:], Sig,
                                         bias=col(_sig_bias_col(j)),
                                         scale=col(_sig_scale_col(j)))

                for i in range(6):
                    terms = TERMS[i]
                    npool = pool_plan.get(i, 0)
                    gi = 0
                    prev = accs[i]
                    for t, (kind, j) in enumerate(terms):
                        if kind == 'd':
                            src, ccol = dts[i], col(_d_col(i))
                        else:
                            src, ccol = vg[j], col(_g_col_of[(i, gi)])
                            gi += 1
                        last = t == len(terms) - 1
                        if t < npool:
                            tmp = midp.tile([P, w], F32, tag=f"tmp{i}",
                                            bufs=2)
                            nc.scalar.activation(
                                tmp[:], src[:],
                                mybir.ActivationFunctionType.Copy,
                                bias=0.0, scale=ccol)
                            # final add writes over the dead d-input slice
                            sacc = dts[i] if last else midp.tile(
                                [P, w], F32, tag=f"sa{i}", bufs=2)
                            nc.gpsimd.tensor_tensor(out=sacc[:], in0=prev[:],
                                                    in1=tmp[:], op=add)
                            prev = sacc
                        else:
                            if last:
                                # write back over the dead d-input slice
                                o = dts[i]
                            elif npool == 0:
                                o = accs[i]     # in-place PSUM accumulate
                            elif t == npool:
                                # first DVE op after a GPSIMD prefix moves
                                # the chain into a PSUM accumulator
                                o = ppool.tile([P, w], F32, tag=f"tp{i}",
                                               bufs=psum_bufs,
                                               name=f"tp{i}")
                            else:
                                o = prev        # in-place PSUM accumulate
                            nc.vector.scalar_tensor_tensor(
                                out=o[:], in0=src[:], scalar=ccol,
                                in1=prev[:], op0=mult, op1=add)
                            prev = o
                if c == nchunk - 1:
                    # last chunk: store in 3-row halves so the early rows
                    # stream out while late rows still compute
                    for lo in (0, 3):
                        nc.scalar.dma_start(
                            out=out[6 + lo:9 + lo, :, sl].rearrange(
                                "r p w -> p r w"),
                            in_=dm[:, lo * w:(lo + 3) * w].rearrange(
                                "p (r w) -> p r w", r=3))
                else:
                    nc.scalar.dma_start(
                        out=out[6:12, :, sl].rearrange("r p w -> p r w"),
                        in_=dm[:].rearrange("p (r w) -> p r w", r=6))
    nc.finalize()
    return nc


def host_coefs(params, bounds_lo, bounds_hi, t, external_drive):
    """All scalar precomputation, float32 throughout to track the fp32
    reference."""
    f = np.float32
    p = np.clip(np.asarray(params, np.float32),
                np.asarray(bounds_lo, np.float32),
                np.asarray(bounds_hi, np.float32)).astype(np.float32)
    t_idx = int(np.clip(f(t) / DT, f(0.0), f(NT - 1)))
    drive = np.asarray(external_drive, np.float32)[t_idx]

    cols = np.zeros(NCOEF, np.float32)
    for j in range(6):
        r = p[RR[j]]
        cols[_sig_scale_col(j)] = r
        cols[_sig_bias_col(j)] = f(-(r * p[V0[j]]))
    for i in range(6):
        g = p[GAIN[i]]
        cols[_d_col(i)] = f(-(f(2.0) * g))
        cols[_k_col(i)] = drive[i]
        cols[_e_col(i)] = f(-(g * g))
        for gi, (pidx, sign) in enumerate(G_COUP[i]):
            j = [j2 for (k2, j2) in TERMS[i] if k2 == 'g'][gi]
            cval = f(p[pidx] * p[GAIN[j]])
            if i != j:
                cval = f(cval * f(p[NN[j]] / p[NN[i]]))
            cval = f(cval * f(f(2.0) * p[E0[j]]))
            cols[_g_col_of[(i, gi)]] = f(sign) * cval
    return np.tile(cols, (P, 1)).astype(np.float32)


_CACHE = {}
LAST_RESULTS = None


def kernel(params, bounds_lo, bounds_hi, t, y, external_drive):
    global LAST_RESULTS
    coefs = host_coefs(params, bounds_lo, bounds_hi, t, external_drive)
    if "nc" not in _CACHE:
        _CACHE["nc"] = _build()
    nc = _CACHE["nc"]

    y_np = np.asarray(y, np.float32)
    in_maps = []
    for k in range(NCORES):
        shard = np.ascontiguousarray(y_np[:, k * BS:(k + 1) * BS]).reshape(
            NROW, P, FTOT)
        in_maps.append({"y": shard, "coef": coefs})

    res = run_bass_kernel_spmd(nc, in_maps, core_ids=list(range(NCORES)))
    LAST_RESULTS = res
    outs = [res.results[k]["out"].reshape(NROW, BS) for k in range(NCORES)]
    return np.concatenate(outs, axis=1)


# revision 34
# speedup vs baseline: 1.1207x; 1.1207x over previous
"""Trainium2 Bass kernel for the CorticalMicrocircuit RHS evaluation.

Computes dy = f(y; params, drive) for B=2,097,152 independent 12-state
neural-mass circuits, data-parallel across 8 NeuronCores (shard batch,
replicate scalars).

Math per batch element (all scalar coefficients precomputed on host):
  out[i]   = y[6+i]                                          i in 0..5
  out[6+i] = K_i + E_i*y[i] + D_i*y[6+i] + sum_j C_ij * G_j
  G_j      = sigmoid(r_j*y[j] - r_j*v0_j)       (2*e0_j folded into C_ij)

Device mapping per core (B_shard = 262144 = 128 partitions x 2048 f32):
  - ACT: per row h_i = E_i*v_i + K_i (affine init, written to the PSUM
    accumulator for DVE rows), then 6 sigmoids computed IN-PLACE over the
    state tiles (dead after the inits read them).
  - DVE: FMA chains acc = src*c + acc with the accumulator in PSUM, so
    each op reads one SBUF operand on the dedicated port -> the shared
    DVE/GpSimd SBUF port pair stays free for GPSIMD.
  - GPSIMD: rows 2 and 5 (mul on tensor_scalar + add on tensor_tensor,
    all in SBUF) run concurrently on the shared port pair.
  - All DMA on HWDGE (nc.sync). Output rows 0..5 are SBUF->DRAM copies
    of the loaded state tiles; computed rows write back over dead input
    tiles to save SBUF.
"""

import numpy as np

import concourse.bacc as bacc
import concourse.mybir as mybir
import concourse.tile as tile
from concourse.bass_utils import run_bass_kernel_spmd

F32 = mybir.dt.float32

# ---- problem geometry (hardcoded per contest contract) ----
B = 2097152
NCORES = 8
BS = B // NCORES          # 262144 per core
P = 128
FTOT = BS // P            # 2048 free elements per partition
NROW = 12
DT = np.float32(0.002)
NT = 50000

# Per output row i: the FMA terms. ('d', i) is the damping term D_i*y[6+i]
# (always first); ('g', j) reads sigmoid tile G_j.
TERMS = [
    [('d', 0), ('g', 2), ('g', 0), ('g', 3), ('g', 4)],
    [('d', 1), ('g', 2), ('g', 0), ('g', 1), ('g', 3), ('g', 4)],
    [('d', 2), ('g', 0), ('g', 1), ('g', 3)],
    [('d', 3), ('g', 0), ('g', 1), ('g', 2), ('g', 3), ('g', 4), ('g', 5)],
    [('d', 4), ('g', 0), ('g', 1), ('g', 2), ('g', 3), ('g', 4), ('g', 5)],
    [('d', 5), ('g', 0), ('g', 3), ('g', 4), ('g', 5)],
]
# coupling param index + sign per 'g' term, aligned with TERMS[i] g-order.
G_COUP = [
    [(18, 1), (0, 1), (27, -1), (37, -1)],
    [(19, 1), (1, 1), (10, 1), (28, -1), (38, -1)],
    [(2, 1), (11, 1), (29, -1)],
    [(3, 1), (12, 1), (21, 1), (30, -1), (39, -1), (46, -1)],
    [(4, 1), (13, 1), (22, 1), (31, -1), (40, -1), (47, -1)],
    [(5, 1), (32, -1), (41, -1), (48, -1)],
]
# param index tables (reference.py unpack order)
GAIN = [6, 14, 23, 33, 42, 49]      # a_p23, a_p5, a_st, b_PV, b_SST, b_VIP
E0 = [7, 15, 24, 34, 43, 50]
V0 = [8, 16, 25, 35, 44, 51]
RR = [9, 17, 26, 36, 45, 52]
NN = [53, 54, 55, 56, 57, 58]

# coef column layout
def _sig_scale_col(j): return 2 * j
def _sig_bias_col(j): return 2 * j + 1
def _d_col(i): return 12 + 2 * i      # D_i = -2*gain_i
def _k_col(i): return 13 + 2 * i      # K_i = drive_i
def _e_col(i): return 24 + i          # E_i = -gain_i^2
_G_COL0 = 30
_g_col_of = {}
_c = _G_COL0
for _i in range(6):
    for _g in range(len(G_COUP[_i])):
        _g_col_of[(_i, _g)] = _c
        _c += 1
NCOEF = 64
assert _c <= NCOEF

# row -> number of leading FMA terms executed on GPSIMD (rest on DVE).
# GPSIMD terms run as ACT mul (tensor_scalar w/ scalar-AP is ~12x slow on
# the Q7 ucode) + GPSIMD tensor_tensor add.
POOL_PLAN = {2: 4, 5: 5}


def _build(ftot=FTOT, w=1024, pool_plan=POOL_PLAN):
    """Build the SPMD program (identical on every core). Bacc (not plain
    Bass): its finalize() splits multi-sem waits into EventSemaphore
    instructions — this walrus accepts at most ONE sync-wait per
    compute/DMA instruction."""
    nc = bacc.Bacc()
    y = nc.declare_dram_parameter("y", [NROW, P, ftot], F32, isOutput=False)
    coef = nc.declare_dram_parameter("coef", [P, NCOEF], F32, isOutput=False)
    out = nc.declare_dram_parameter("out", [NROW, P, ftot], F32, isOutput=True)
    nchunk = ftot // w
    banks_per_acc = (4 * w + 2047) // 2048
    # every row whose DVE tail has >=2 ops holds a PSUM accumulator
    n_psum_rows = sum(1 for i in range(6)
                      if len(TERMS[i]) - pool_plan.get(i, 0) >= 2)
    psum_bufs = max(1, 8 // max(1, n_psum_rows * banks_per_acc))

    mult = mybir.AluOpType.mult
    add = mybir.AluOpType.add
    Sig = mybir.ActivationFunctionType.Sigmoid
    Ident = mybir.ActivationFunctionType.Identity

    with tile.TileContext(nc) as tc:
        with (
            tc.tile_pool(name="const", bufs=1) as cpool,
            tc.tile_pool(name="io", bufs=2) as iop,
            tc.tile_pool(name="mid", bufs=2) as midp,
            tc.tile_pool(name="acc", bufs=1, space="PSUM") as ppool,
        ):
            ct = cpool.tile([P, NCOEF], F32)
            nc.sync.dma_start(out=ct[:], in_=coef[:, :])
            # one tiny coef read per compute engine right after the load:
            # advances each engine's clock past the coef DMA lane so real
            # ops don't each need an extra wait on it.
            scr = cpool.tile([P, 4], F32)
            nc.scalar.copy(scr[:, 0:1], ct[:, 0:1])
            nc.vector.tensor_copy(scr[:, 1:2], ct[:, 0:1])
            nc.gpsimd.tensor_copy(scr[:, 2:3], ct[:, 0:1])

            def col(c0):
                return ct[:, c0:c0 + 1]

            for c in range(nchunk):
                sl = slice(c * w, (c + 1) * w)
                # one merged DMA per 6-row group: 3 MB transfers, far fewer
                # dispatches and cross-engine semaphores than per-row DMAs
                vm = iop.tile([P, 6 * w], F32, tag="vm", bufs=2)
                dm = iop.tile([P, 6 * w], F32, tag="dm", bufs=2)
                if c == 0:
                    # first chunk: load in 3-row halves so ACT starts as
                    # soon as the first 1.5 MB lands (shorter pipe fill)
                    for lo in (0, 3):
                        nc.sync.dma_start(
                            out=vm[:, lo * w:(lo + 3) * w].rearrange(
                                "p (r w) -> p r w", r=3),
                            in_=y[lo:lo + 3, :, sl].rearrange(
                                "r p w -> p r w"))
                        nc.sync.dma_start(
                            out=dm[:, lo * w:(lo + 3) * w].rearrange(
                                "p (r w) -> p r w", r=3),
                            in_=y[6 + lo:9 + lo, :, sl].rearrange(
                                "r p w -> p r w"))
                else:
                    nc.sync.dma_start(
                        out=vm[:].rearrange("p (r w) -> p r w", r=6),
                        in_=y[0:6, :, sl].rearrange("r p w -> p r w"))
                    nc.sync.dma_start(
                        out=dm[:].rearrange("p (r w) -> p r w", r=6),
                        in_=y[6:12, :, sl].rearrange("r p w -> p r w"))
                # out rows 0..5 are a plain copy of state rows 6..11
                nc.sync.dma_start(
                    out=out[0:6, :, sl].rearrange("r p w -> p r w"),
                    in_=dm[:].rearrange("p (r w) -> p r w", r=6))
                vg = [vm[:, j * w:(j + 1) * w] for j in range(6)]
                dts = [dm[:, i * w:(i + 1) * w] for i in range(6)]

                # h_i = E_i*v_i + K_i (ACT, into SBUF). Emission order =
                # demand order: DVE rows' h first so their d-terms start
                # early; all h before any in-place sigmoid (h reads v).
                accs = [None] * 6
                h_order = [i for i in range(6) if pool_plan.get(i, 0) == 0]
                h_order += [i for i in range(6) if pool_plan.get(i, 0) > 0]
                for i in h_order:
                    if pool_plan.get(i, 0) > 0:
                        h = midp.tile([P, w], F32, tag=f"hs{i}", bufs=2,
                                      name=f"hs{i}")
                    else:
                        # DVE rows: init straight into the PSUM accumulator
                        h = ppool.tile([P, w], F32, tag=f"hp{i}",
                                       bufs=psum_bufs, name=f"hp{i}")
                    nc.scalar.activation(h[:], vg[i][:], Ident,
                                         bias=col(_k_col(i)),
                                         scale=col(_e_col(i)))
                    accs[i] = h
                # sigmoids, in place over the (now dead) state tiles,
                # ordered by first use in the DVE rows' chains
                for j in (2, 0, 1, 3, 4, 5):
                    nc.scalar.activation(vg[j][:], vg[j][:], Sig,
                                         bias=col(_sig_bias_col(j)),
                                         scale=col(_sig_scale_col(j)))

                for i in range(6):
                    terms = TERMS[i]
                    npool = pool_plan.get(i, 0)
                    gi = 0
                    prev = accs[i]
                    for t, (kind, j) in enumerate(terms):
                        if kind == 'd':
                            src, ccol = dts[i], col(_d_col(i))
                        else:
                            src, ccol = vg[j], col(_g_col_of[(i, gi)])
                            gi += 1
                        last = t == len(terms) - 1
                        if t < npool:
                            tmp = midp.tile([P, w], F32, tag=f"tmp{i}",
                                            bufs=2)
                            nc.scalar.activation(
                                tmp[:], src[:],
                                mybir.ActivationFunctionType.Copy,
                                bias=0.0, scale=ccol)
                            # final add writes over the dead d-input slice
                            sacc = dts[i] if last else midp.tile(
                                [P, w], F32, tag=f"sa{i}", bufs=2)
                            nc.gpsimd.tensor_tensor(out=sacc[:], in0=prev[:],
                                                    in1=tmp[:], op=add)
                            prev = sacc
                        else:
                            if last:
                                # write back over the dead d-input slice
                                o = dts[i]
                            elif npool == 0:
                                o = accs[i]     # in-place PSUM accumulate
                            elif t == npool:
                                # first DVE op after a GPSIMD prefix moves
                                # the chain into a PSUM accumulator
                                o = ppool.tile([P, w], F32, tag=f"tp{i}",
                                               bufs=psum_bufs,
                                               name=f"tp{i}")
                            else:
                                o = prev        # in-place PSUM accumulate
                            nc.vector.scalar_tensor_tensor(
                                out=o[:], in0=src[:], scalar=ccol,
                                in1=prev[:], op0=mult, op1=add)
                            prev = o
                if c == nchunk - 1:
                    # last chunk: store in 3-row halves so the early rows
                    # stream out while late rows still compute
                    for lo in (0, 3):
                        nc.sync.dma_start(
                            out=out[6 + lo:9 + lo, :, sl].rearrange(
                                "r p w -> p r w"),
                            in_=dm[:, lo * w:(lo + 3) * w].rearrange(
                                "p (r w) -> p r w", r=3))
                else:
                    nc.sync.dma_start(
                        out=out[6:12, :, sl].rearrange("r p w -> p r w"),
                        in_=dm[:].rearrange("p (r w) -> p r w", r=6))
    nc.finalize()
    return nc


def host_coefs(params, bounds_lo, bounds_hi, t, external_drive):
    """All scalar precomputation, float32 throughout to track the fp32
    reference."""
    f = np.float32
    p = np.clip(np.asarray(params, np.float32),
                np.asarray(bounds_lo, np.float32),
                np.asarray(bounds_hi, np.float32)).astype(np.float32)
    t_idx = int(np.clip(f(t) / DT, f(0.0), f(NT - 1)))
    drive = np.asarray(external_drive, np.float32)[t_idx]

    cols = np.zeros(NCOEF, np.float32)
    for j in range(6):
        r = p[RR[j]]
        cols[_sig_scale_col(j)] = r
        cols[_sig_bias_col(j)] = f(-(r * p[V0[j]]))
    for i in range(6):
        g = p[GAIN[i]]
        cols[_d_col(i)] = f(-(f(2.0) * g))
        cols[_k_col(i)] = drive[i]
        cols[_e_col(i)] = f(-(g * g))
        for gi, (pidx, sign) in enumerate(G_COUP[i]):
            j = [j2 for (k2, j2) in TERMS[i] if k2 == 'g'][gi]
            cval = f(p[pidx] * p[GAIN[j]])
            if i != j:
                cval = f(cval * f(p[NN[j]] / p[NN[i]]))
            cval = f(cval * f(f(2.0) * p[E0[j]]))
            cols[_g_col_of[(i, gi)]] = f(sign) * cval
    return np.tile(cols, (P, 1)).astype(np.float32)


_CACHE = {}
LAST_RESULTS = None


def kernel(params, bounds_lo, bounds_hi, t, y, external_drive):
    global LAST_RESULTS
    coefs = host_coefs(params, bounds_lo, bounds_hi, t, external_drive)
    if "nc" not in _CACHE:
        _CACHE["nc"] = _build()
    nc = _CACHE["nc"]

    y_np = np.asarray(y, np.float32)
    in_maps = []
    for k in range(NCORES):
        shard = np.ascontiguousarray(y_np[:, k * BS:(k + 1) * BS]).reshape(
            NROW, P, FTOT)
        in_maps.append({"y": shard, "coef": coefs})

    res = run_bass_kernel_spmd(nc, in_maps, core_ids=list(range(NCORES)))
    LAST_RESULTS = res
    outs = [res.results[k]["out"].reshape(NROW, BS) for k in range(NCORES)]
    return np.concatenate(outs, axis=1)
